# revision 1
# baseline (speedup 1.0000x reference)
"""AAConv (attention-augmented conv) Trainium2 kernel, 8-core data-parallel.

Reference shapes: x (16,256,32,32) f32
  conv branch: 3x3 SAME conv 256->128 (+bias)
  attn branch: 1x1 qkv conv (k|q|v = 128|128|128 rows of qkv_w), 8 heads d=16,
               softmax attention over 1024 positions, 1x1 proj 128->128 (+bias)
  out = concat([conv_out, attn_out], axis=1) -> (16,256,32,32)

Sharding: pure data-parallel over batch. Each of 8 cores gets 2 images and
all weights; outputs concatenated on host.

Per-core design (channels on partitions, pixels on free dim):
 - logits computed transposed, L^T[k,q] (lhsT=K_h [16,128], rhs=Q_h [16,512]),
   4-way row-tiled over heads (K=16 contraction, tile_position=(32h,0)).
   K/Q stored padded: head h at partitions 32h..32h+16 (zero pad rows) so
   lhsT/rhs share base partitions; produced by M=128 matmuls against
   zero-padded transposed weights.
 - softmax denominator via a ones-column appended to V^T in the AV matmul
   (M=17 col-tiled, tile_position=(0,32h)): no cross-partition reductions.
 - exp on ScalarE (the bottleneck engine, ~110us/core floor) straight out
   of PSUM in [128,1024] chunks.
 - softmax normalize: full-tile reciprocal on DVE, per-quadrant broadcast of
   1/den via K=1 bf16 matmuls on independent PE tiles, one full-tile
   multiply; attn kept in the padded partition layout and proj done with
   zero-padded transposed proj weights (pad rows exactly 0 so junk rows
   never reach the output).
 - conv rhs as single contiguous runs over a zero-padded 34-wide flat
   space, in 15/15/2-row chunks (matmul rhs APs must be 1-D); junk columns
   skipped on evacuation.
 - all matmuls bf16 (f32 psum accumulate). PSUM: lg 3x2 banks (lookahead
   for the exp pipeline) + av 1 + ms 1.
 - measured ~204us steady-state per 2-image workload on HW (big-rep slope);
   TimelineSim model 205us; ScalarE-exp is the floor at ~134-146us.
"""

import sys

for p in ("/opt/trn_rl_repo",):
    if p not in sys.path:
        sys.path.insert(0, p)

import numpy as np

import concourse.tile as tile
from concourse import bacc, mybir
from concourse.masks import make_identity

F32 = mybir.dt.float32
BF16 = mybir.dt.bfloat16
I16 = mybir.dt.int16
AF = mybir.ActivationFunctionType
ALU = mybir.AluOpType

# Problem dims (hardcoded)
B, C, H, W = 16, 256, 32, 32
HW = H * W                      # 1024
CO, DK, DV, NH = 256, 128, 128, 8
D = DK // NH                    # 16 head dim
CONV_CO = CO - DV               # 128
N_CORES = 8
BL = B // N_CORES               # 2 images per core
HP = H + 2                      # 34 padded
PADHW = HP * HP                 # 1156
SCALE = float(D) ** -0.5        # 0.25
EXA = 128.0 / 0.6931471805599453 * SCALE
EXB = 16249.5
import os as _os_
DVE_KTS = set(int(v) for v in _os_.environ.get(
    "AACONV_DVE_KTS", "0,2,4,5,7").split(",") if v != "")


def build_nc():
    nc = bacc.Bacc("TRN2", target_bir_lowering=False, debug=False,
                   num_devices=N_CORES)

    x_ext = nc.declare_dram_parameter("x", [BL, C, HW], F32, isOutput=False)
    convw_ext = nc.declare_dram_parameter("conv_w", [9, C, CONV_CO], F32, isOutput=False)
    convb_ext = nc.declare_dram_parameter("conv_b", [1, CONV_CO], F32, isOutput=False)
    qkvw_ext = nc.declare_dram_parameter("qkv_w", [2 * DK + DV, C], F32, isOutput=False)
    qkvb_ext = nc.declare_dram_parameter("qkv_b", [1, 2 * DK + DV], F32, isOutput=False)
    projw_ext = nc.declare_dram_parameter("proj_w", [DV, DV], F32, isOutput=False)
    projb_ext = nc.declare_dram_parameter("proj_b", [1, DV], F32, isOutput=False)
    out_ext = nc.declare_dram_parameter("out", [BL, CO, HW], F32, isOutput=True)

    with tile.TileContext(nc) as tc:
        with (
            tc.tile_pool(name="const", bufs=1) as constp,
            tc.tile_pool(name="stage", bufs=1) as stagep,
            tc.tile_pool(name="img", bufs=3) as imgp,
            tc.tile_pool(name="st", bufs=6) as stp,
            tc.tile_pool(name="psum", bufs=1, space="PSUM") as psp,
        ):
            # ---------------- weights ----------------
            ident = constp.tile([128, 128], F32)
            make_identity(nc, ident[:])

            # qkv weights: DMA natural [chan, c], PE-transpose to [c, chan],
            # scatter into zero-padded layouts.
            qkvw_sb = stagep.tile([128, 3 * C], F32)  # blk b at cols b*256
            for blk in range(3):
                nc.sync.dma_start(
                    qkvw_sb[:, blk * C:(blk + 1) * C],
                    qkvw_ext[blk * 128:(blk + 1) * 128, :],
                )

            # prefetch all images' x early (longest DMA pole) + compact bf16
            import os as _os
            _reps = int(_os.environ.get("AACONV_BENCH_REPS", "1"))
            xs_bf = []
            for img in range(BL * _reps):
                xin = imgp.tile([128, 2 * HW], F32, tag="xin", name=f"xin_{img}")
                for ct in range(2):
                    nc.sync.dma_start(
                        xin[:, ct * HW:(ct + 1) * HW],
                        x_ext[img % BL, ct * 128:(ct + 1) * 128, :])
                x_bf = imgp.tile([128, 2 * HW], BF16, tag="xbf", name=f"xbf_{img}")
                nc.gpsimd.tensor_copy(x_bf[:], xin[:])
                xs_bf.append(x_bf)

            # wkq_pad: [128 c, ct*512 + tgt*256 + hh*128 + hp*32 + d] bf16, zero pad
            wkq_pad = constp.tile([128, 2 * 512], BF16)
            nc.gpsimd.memset(wkq_pad[:], 0.0)
            wvT = constp.tile([128, 2 * 128], BF16)
            for ct in range(2):
                for blk in range(3):
                    tps = psp.tile([128, 128], F32, tag="ms", name=f"tps_{ct}_{blk}")
                    nc.tensor.transpose(
                        tps[:], qkvw_sb[:, blk * C + ct * 128: blk * C + (ct + 1) * 128],
                        ident[:])
                    if blk < 2:
                        dst = wkq_pad[:, ct * 512 + blk * 256:
                                      ct * 512 + (blk + 1) * 256].rearrange(
                            "p (h d) -> p h d", d=32)[:, :, 0:16]
                        src = tps[:].rearrange("p (h d) -> p h d", d=16)
                        nc.vector.tensor_copy(dst, src)
                    else:
                        nc.vector.tensor_copy(
                            wvT[:, ct * 128:(ct + 1) * 128], tps[:])

            projw_sb = stagep.tile([128, 128], F32)
            nc.sync.dma_start(projw_sb[:], projw_ext[:])
            # padded projT: rows 32hp+16+d = proj_w^T row (4hh+hp)*16+d, rest 0
            # (matches the attn_pad layout where attn lives at rows 32hp+16..32).
            # Column-scatter proj_w in free space first, then PE-transpose.
            projw_pad = stagep.tile([128, 2 * 128], F32)
            nc.gpsimd.memset(projw_pad[:], 0.0)
            for hh in range(2):
                nc.vector.tensor_copy(
                    projw_pad[:, hh * 128:(hh + 1) * 128].rearrange(
                        "p (a b) -> p a b", b=32)[:, :, 16:32],
                    projw_sb[:, 64 * hh:64 * (hh + 1)].rearrange(
                        "p (a b) -> p a b", b=16))
            projT_pad = constp.tile([128, 2 * 128], BF16)
            for hh in range(2):
                tps2 = psp.tile([128, 128], F32, tag="ms", name=f"tps2_{hh}")
                nc.tensor.transpose(
                    tps2[:], projw_pad[:, hh * 128:(hh + 1) * 128], ident[:])
                nc.vector.tensor_copy(projT_pad[:, hh * 128:(hh + 1) * 128],
                                      tps2[:])

            # biases as per-partition COLUMNS (partition-scattered by DMA),
            # folded into the PSUM evacuations as tensor_scalar adds - no
            # bias matmuls on the PE.
            # bcol_kq[:, tgt*2+hh]: rows hp*32+d = qkv_b[tgt*128+(hh*4+hp)*16+d]

            # v-bias broadcast to 128 partitions via PE (ones ⊗ bv)
            bv_f32 = stagep.tile([1, DV], F32)
            nc.sync.dma_start(bv_f32[:], qkvb_ext[:, 2 * DK:])
            bv_bf = stagep.tile([1, DV], BF16)
            nc.vector.tensor_copy(bv_bf[:], bv_f32[:])
            ones_row = constp.tile([1, 128], BF16)
            nc.gpsimd.memset(ones_row[:], 1.0)
            ones_q = constp.tile([128, 32], BF16)
            nc.gpsimd.memset(ones_q[:], 1.0)
            pre_ps = psp.tile([128, 512], F32, tag="ms", name="pre_ps")
            nc.tensor.matmul(pre_ps[:, 384:512], ones_row[:], bv_bf[:],
                             start=True, stop=True, skip_group_check=True)
            bv_bc = constp.tile([128, 128], F32)
            nc.vector.tensor_copy(bv_bc[:], pre_ps[:, 384:512])

            # conv weights: natural [c, o] per tap, bf16. cols (ct*9+t)*128+o
            wconv_f32 = stagep.tile([128, 2 * 9 * CONV_CO], F32)
            wconv = constp.tile([128, 2 * 9 * CONV_CO], BF16)
            for ct in range(2):
                for t in range(9):
                    blk = slice((ct * 9 + t) * 128, (ct * 9 + t + 1) * 128)
                    nc.sync.dma_start(wconv_f32[:, blk],
                                      convw_ext[t, ct * 128:(ct + 1) * 128, :])
                    nc.vector.tensor_copy(wconv[:, blk], wconv_f32[:, blk])

            # --- bias columns + quadmask, built via PE (no SBUF-writing DMAs)
            # bias ROWS first (free-dim scatters, DVE-legal), then a K=1
            # matmul against ones[1,1] turns each row into a column.
            qkvb_sb = stagep.tile([1, 2 * DK + DV], F32)
            nc.sync.dma_start(qkvb_sb[:], qkvb_ext[:])
            brow_pad = constp.tile([1, 512], BF16)
            nc.gpsimd.memset(brow_pad[:], 0.0)
            for tgt in range(2):
                for hh in range(2):
                    nc.vector.tensor_copy(
                        brow_pad[0:1, (tgt * 2 + hh) * 128:
                                 (tgt * 2 + hh + 1) * 128].rearrange(
                            "p (a b) -> p a b", b=32)[:, :, 0:16],
                        qkvb_sb[0:1, tgt * DK + 64 * hh: tgt * DK + 64 * (hh + 1)
                                ].rearrange("p (a b) -> p a b", b=16))
            convb_f32 = stagep.tile([1, CONV_CO], F32)
            nc.sync.dma_start(convb_f32[:], convb_ext[:])
            convb_row = constp.tile([1, CONV_CO], BF16)
            nc.vector.tensor_copy(convb_row[:], convb_f32[:])
            projb_f32 = stagep.tile([1, DV], F32)
            nc.sync.dma_start(projb_f32[:], projb_ext[:])
            projb_row = constp.tile([1, DV], BF16)
            nc.vector.tensor_copy(projb_row[:], projb_f32[:])
            ones11 = constp.tile([1, 1], BF16)
            nc.gpsimd.memset(ones11[:], 1.0)
            for blk in range(4):
                nc.tensor.matmul(pre_ps[:, blk:blk + 1],
                                 brow_pad[0:1, blk * 128:(blk + 1) * 128],
                                 ones11[0:1, :], start=True, stop=True,
                                 skip_group_check=True)
            nc.tensor.matmul(pre_ps[:, 4:5], convb_row[0:1, :], ones11[0:1, :],
                             start=True, stop=True, skip_group_check=True)
            nc.tensor.matmul(pre_ps[:, 5:6], projb_row[0:1, :], ones11[0:1, :],
                             start=True, stop=True, skip_group_check=True)
            bias_cols = constp.tile([128, 6], F32)
            nc.vector.tensor_copy(bias_cols[:], pre_ps[:, 0:6])
            # quadmask [4,128] (row r ones on cols 32r..32r+32) via transpose
            maskT = stagep.tile([128, 128], F32)
            nc.vector.memset(maskT[:], 0.0)
            for r in range(4):
                nc.vector.memset(maskT[32 * r:32 * (r + 1), r:r + 1], 1.0)
            nc.tensor.transpose(pre_ps[:, 128:256], maskT[:], ident[:])
            quadmask = constp.tile([4, 128], BF16)
            nc.vector.tensor_copy(quadmask[:], pre_ps[0:4, 128:256])

            # ---------------- per image ----------------
            PADW = PADHW + 36   # room for the last conv chunk's shifted reads
            for img in range(BL * _reps):
                imgd = img % BL
                x_bf = xs_bf[img]
                # zero-padded 34x34 layout for the conv, filled via DMA
                xpad = imgp.tile([128, 2 * PADW], BF16, tag="xpad", name=f"xpad_{img}")
                nc.gpsimd.memset(xpad[:], 0.0)
                for ct in range(2):
                    nc.sync.dma_start(
                        xpad[:, ct * PADW: ct * PADW + PADHW].rearrange(
                            "p (h w) -> p h w", h=HP)[:, 1:33, 1:33],
                        x_bf[:, ct * HW:(ct + 1) * HW].rearrange(
                            "p (h w) -> p h w", h=H))

                # ---- K_pad / Q_pad ----
                k_pad = imgp.tile([128, 2 * HW], BF16, tag="kpad", name=f"kpad_{img}")
                q_pad = imgp.tile([128, 2 * HW], BF16, tag="qpad", name=f"qpad_{img}")
                for hh in range(2):
                    for tgt, dst in ((0, k_pad), (1, q_pad)):
                        for qn in range(2):
                            kqps = psp.tile([128, 512], F32, tag="ms",
                                            name=f"kqps_{img}_{tgt}_{hh}_{qn}")
                            for ct in range(2):
                                nc.tensor.matmul(
                                    kqps[:],
                                    wkq_pad[:, ct * 512 + tgt * 256 + hh * 128:
                                            ct * 512 + tgt * 256 + (hh + 1) * 128],
                                    x_bf[:, ct * HW + qn * 512:
                                         ct * HW + (qn + 1) * 512],
                                    start=(ct == 0), stop=(ct == 1))
                            nc.vector.tensor_scalar(
                                dst[:, hh * HW + qn * 512:
                                    hh * HW + (qn + 1) * 512], kqps[:],
                                bias_cols[:, tgt * 2 + hh: tgt * 2 + hh + 1],
                                None, ALU.add)

                # ---- V^T with ones column, 32-stride padded blocks ----
                # vt_aug block (hh,kt) at cols (hh*8+kt)*128 + hp*32 +
                #   [0 = ones, 1:16 = zeros, 16:32 = V_h]  (M=32 AV matmuls
                #   write full PSUM quadrants; denominator lands on quadrant
                #   rows 32hp, attn on rows 32hp+16..32)
                vt_aug = imgp.tile([128, 2 * 8 * 128], BF16, tag="vtaug",
                                   name=f"vtaug_{img}")
                # pad value 1e-4 (not 0) keeps the reciprocal of pad rows
                # finite; proj weights for pad rows are exactly 0 so the
                # values never reach the output
                nc.gpsimd.memset(vt_aug[:], 1e-4)
                nc.gpsimd.memset(
                    vt_aug[:].rearrange("p (g d) -> p g d", d=32)[:, :, 0:1], 1.0)
                for kt in range(8):
                    vtps = psp.tile([128, 128], F32, tag="ms", name=f"vtps_{img}_{kt}")
                    for ct in range(2):
                        nc.tensor.matmul(
                            vtps[:],
                            x_bf[:, ct * HW + kt * 128: ct * HW + (kt + 1) * 128],
                            wvT[:, ct * 128:(ct + 1) * 128],
                            start=(ct == 0), stop=(ct == 1))
                    for hh in range(2):
                        base = (hh * 8 + kt) * 128
                        dst = vt_aug[:, base: base + 128].rearrange(
                            "p (h d) -> p h d", d=32)[:, :, 16:32]
                        src = vtps[:, hh * 64:(hh + 1) * 64].rearrange(
                            "p (h d) -> p h d", d=16)
                        bvb = bv_bc[:, hh * 64:(hh + 1) * 64].rearrange(
                            "p (h d) -> p h d", d=16)
                        nc.vector.tensor_add(dst, src, bvb)

                # ---- conv branch ----
                # computed over the padded flat space in row-aligned chunks so
                # every matmul rhs is a single contiguous run; junk columns
                # (x=32,33 of each padded row) are skipped on evacuation.
                out_conv = imgp.tile([128, HW], F32, tag="oconv", name=f"oconv_{img}")
                for (r0, nr) in ((0, 15), (15, 15), (30, 2)):
                    n = (nr - 1) * HP + W          # chunk free size (<=512)
                    cs = (r0 + 1) * HP + 1         # pad-flat offset of (r0, 0)
                    cvps = psp.tile([128, 512], F32, tag="ms",
                                    name=f"cvps_{img}_{r0}")
                    for t in range(9):
                        dy, dx = t // 3, t % 3
                        sh = (dy - 1) * HP + (dx - 1)
                        for ct in range(2):
                            nc.tensor.matmul(
                                cvps[:, 0:n],
                                wconv[:, (ct * 9 + t) * 128:(ct * 9 + t + 1) * 128],
                                xpad[:, ct * PADW + cs + sh: ct * PADW + cs + sh + n],
                                start=((t, ct) == (0, 0)), stop=((t, ct) == (8, 1)))
                    nc.vector.tensor_scalar(
                        out_conv[:, r0 * W:(r0 + nr) * W].rearrange(
                            "p (h w) -> p h w", h=nr),
                        cvps[:, 0:nr * HP].rearrange(
                            "p (h w) -> p h w", w=HP)[:, :, 0:W],
                        bias_cols[:, 4:5], None, ALU.add)
                nc.sync.dma_start(out_ext[imgd, 0:CONV_CO, :], out_conv[:])

                # ---- attention ----
                # attn_pad: [128, 2*HW] bf16, hh at cols hh*HW; head hp's
                # normalized attn at rows 32hp+16..32 (fully written by the
                # normalize muls, so no memset needed)
                attn_pad = imgp.tile([128, 2 * HW], BF16, tag="attnp",
                                     name=f"attnp_{img}")
                rrec = imgp.tile([128, HW], F32, tag="rrec", name=f"rrec_{img}")
                rrec_bf = imgp.tile([128, HW], BF16, tag="rrecbf",
                                    name=f"rrecbf_{img}")
                # qh-serial: av is one PSUM bank, which frees banks for
                # lg bufs=3 (a full lookahead tile -> no ACT ping-pong stall)
                def normalize(av_sb, sl, slh, tag):
                    # av rows per quadrant hp: 32hp = den, +1..16 = pad,
                    # +16..32 = unnormalized attn. Full-tile ops only;
                    # broadcast 1/den per quadrant via K=1 bf16 matmuls on
                    # independent (row,col) PE tiles.
                    nc.vector.reciprocal_approx_fast(rrec[:, sl], av_sb[:])
                    nc.vector.tensor_copy(rrec_bf[:, sl], rrec[:, sl])
                    rd4 = imgp.tile([4, 512], BF16, tag="rd4",
                                    name=f"rd4_{tag}")
                    nc.sync.dma_start(
                        rd4[:],
                        rrec_bf[:, sl].rearrange(
                            "(h e) n -> h e n", e=32)[:, 0, :])
                    rdps = psp.tile([128, 512], F32, tag="ms",
                                    name=f"rdps_{tag}")
                    nc.tensor.matmul(rdps[:], quadmask[0:4, :], rd4[:],
                                     start=True, stop=True)
                    nc.vector.tensor_mul(attn_pad[:, slh], av_sb[:], rdps[:])

                pending = None   # defer each round's normalize tail past the
                                 # next round's first iteration (keeps PE on
                                 # logits at round boundaries)
                for hh in range(2):
                    for qh in range(2):
                        sl = slice(qh * 512, (qh + 1) * 512)
                        slh = slice(hh * HW + qh * 512, hh * HW + (qh + 1) * 512)
                        av = psp.tile([128, 512], F32, tag="av", bufs=1,
                                      name=f"av_{img}_{hh}_{qh}")
                        for kt in range(8):
                            lgs = []
                            for hg in range(2):
                                lg = psp.tile([128, 1024], F32, tag="lg", bufs=3,
                                              name=f"lg_{img}_{hh}_{qh}_{kt}_{hg}")
                                lgs.append(lg)
                                for j in range(2):
                                    hp = 2 * hg + j
                                    nc.tensor.matmul(
                                        lg[:, j * 512:(j + 1) * 512],
                                        k_pad[32 * hp:32 * hp + 16,
                                              hh * HW + kt * 128: hh * HW + (kt + 1) * 128],
                                        q_pad[32 * hp:32 * hp + 16,
                                              hh * HW + qh * 512: hh * HW + (qh + 1) * 512],
                                        start=True, stop=True,
                                        tile_position=(32 * hp, 0))
                            sts = []
                            for hg in range(2):
                                st = stp.tile([128, 1024], BF16, tag="st",
                                              name=f"st_{img}_{hh}_{qh}_{kt}_{hg}")
                                sts.append(st)
                                if hg == 0 or kt not in DVE_KTS:
                                    nc.scalar.activation(st[:], lgs[hg][:], AF.Exp,
                                                         scale=SCALE)
                                else:
                                    nc.vector.tensor_scalar(
                                        st[:].bitcast(I16), lgs[hg][:],
                                        EXA, EXB, ALU.mult, ALU.add)
                            for hg in range(2):
                                for j in range(2):
                                    hp = 2 * hg + j
                                    base = (hh * 8 + kt) * 128 + 32 * hp
                                    nc.tensor.matmul(
                                        av[32 * hp:32 * hp + 32, :],
                                        vt_aug[:, base: base + 32],
                                        sts[hg][:, j * 512:(j + 1) * 512],
                                        start=(kt == 0), stop=(kt == 7),
                                        skip_group_check=True,
                                        tile_position=(0, 32 * hp))
                            if kt == 0 and pending is not None:
                                normalize(*pending)
                                pending = None
                        # evacuate av to SBUF right away so the PSUM bank
                        # frees; defer the rest of the normalize.
                        av_sb = imgp.tile([128, 512], F32, tag="avsb",
                                          name=f"avsb_{img}_{hh}_{qh}")
                        nc.vector.tensor_copy(av_sb[:], av[:])
                        pending = (av_sb, sl, slh, f"{img}_{hh}_{qh}")
                if pending is not None:
                    normalize(*pending)
                    pending = None

                # ---- proj (padded weights over both halves) ----
                out_proj = imgp.tile([128, HW], F32, tag="oproj", name=f"oproj_{img}")
                for qn in range(2):
                    projps = psp.tile([128, 512], F32, tag="ms",
                                      name=f"projps_{img}_{qn}")
                    for hh in range(2):
                        nc.tensor.matmul(
                            projps[:],
                            projT_pad[:, hh * 128:(hh + 1) * 128],
                            attn_pad[:, hh * HW + qn * 512: hh * HW + (qn + 1) * 512],
                            start=(hh == 0), stop=(hh == 1))
                    nc.vector.tensor_scalar(
                        out_proj[:, qn * 512:(qn + 1) * 512], projps[:],
                        bias_cols[:, 5:6], None, ALU.add)
                nc.sync.dma_start(out_ext[imgd, CONV_CO:, :], out_proj[:])

    return nc


_NC = None


def _get_nc():
    global _NC
    if _NC is None:
        _NC = build_nc()
        _NC.compile()
    return _NC


def kernel(**inputs):
    from concourse.bass_utils import run_bass_kernel_spmd

    nc = _get_nc()
    x = np.asarray(inputs["x"], np.float32).reshape(B, C, HW)
    conv_w = np.ascontiguousarray(np.asarray(inputs["conv_w"], np.float32).reshape(9, C, CONV_CO))
    conv_b = np.ascontiguousarray(np.asarray(inputs["conv_b"], np.float32).reshape(1, CONV_CO))
    qkv_w = np.ascontiguousarray(np.asarray(inputs["qkv_w"], np.float32))
    qkv_b = np.ascontiguousarray(np.asarray(inputs["qkv_b"], np.float32).reshape(1, 2 * DK + DV))
    proj_w = np.ascontiguousarray(np.asarray(inputs["proj_w"], np.float32))
    proj_b = np.ascontiguousarray(np.asarray(inputs["proj_b"], np.float32).reshape(1, DV))

    in_maps = []
    for i in range(N_CORES):
        in_maps.append({
            "x": np.ascontiguousarray(x[i * BL:(i + 1) * BL]),
            "conv_w": conv_w, "conv_b": conv_b,
            "qkv_w": qkv_w, "qkv_b": qkv_b,
            "proj_w": proj_w, "proj_b": proj_b,
        })
    res = run_bass_kernel_spmd(nc, in_maps, core_ids=list(range(N_CORES)))
    outs = [np.asarray(res.results[i]["out"]).reshape(BL, CO, H, W)
            for i in range(N_CORES)]
    return np.concatenate(outs, axis=0).astype(np.float32)


if __name__ == "__main__":
    nc = build_nc()
    nc.compile()
    print("built ok; instructions:", len(nc.inst_map))



# revision 12
# speedup vs baseline: 3.6354x; 3.6354x over previous
"""AAConv (attention-augmented conv) Trainium2 kernel, 8-core data-parallel.

Reference shapes: x (16,256,32,32) f32
  conv branch: 3x3 SAME conv 256->128 (+bias)
  attn branch: 1x1 qkv conv (k|q|v = 128|128|128 rows of qkv_w), 8 heads d=16,
               softmax attention over 1024 positions, 1x1 proj 128->128 (+bias)
  out = concat([conv_out, attn_out], axis=1) -> (16,256,32,32)

Sharding: pure data-parallel over batch. Each of 8 cores gets 2 images and
all weights; outputs concatenated on host.

Per-core design (channels on partitions, pixels on free dim):
 - logits computed transposed, L^T[k,q] (lhsT=K_h [16,128], rhs=Q_h [16,512]),
   4-way row-tiled over heads (K=16 contraction, tile_position=(32h,0)).
   K/Q stored padded: head h at partitions 32h..32h+16 (zero pad rows) so
   lhsT/rhs share base partitions; produced by M=128 matmuls against
   zero-padded transposed weights.
 - softmax denominator via a ones-column appended to V^T in the AV matmul
   (M=17 col-tiled, tile_position=(0,32h)): no cross-partition reductions.
 - exp on ScalarE (the bottleneck engine, ~110us/core floor) straight out
   of PSUM in [128,1024] chunks.
 - softmax normalize: full-tile reciprocal on DVE, per-quadrant broadcast of
   1/den via K=1 bf16 matmuls on independent PE tiles, one full-tile
   multiply; attn kept in the padded partition layout and proj done with
   zero-padded transposed proj weights (pad rows exactly 0 so junk rows
   never reach the output).
 - conv rhs as single contiguous runs over a zero-padded 34-wide flat
   space, in 15/15/2-row chunks (matmul rhs APs must be 1-D); junk columns
   skipped on evacuation.
 - all matmuls bf16 (f32 psum accumulate). PSUM: lg 3x2 banks (lookahead
   for the exp pipeline) + av 1 + ms 1.
 - measured ~204us steady-state per 2-image workload on HW (big-rep slope);
   TimelineSim model 205us; ScalarE-exp is the floor at ~134-146us.
"""

import sys

for p in ("/opt/trn_rl_repo",):
    if p not in sys.path:
        sys.path.insert(0, p)

import numpy as np

import concourse.tile as tile
from concourse import bacc, mybir
from concourse.masks import make_identity

F32 = mybir.dt.float32
BF16 = mybir.dt.bfloat16
I16 = mybir.dt.int16
AF = mybir.ActivationFunctionType
ALU = mybir.AluOpType

# Problem dims (hardcoded)
B, C, H, W = 16, 256, 32, 32
HW = H * W                      # 1024
CO, DK, DV, NH = 256, 128, 128, 8
D = DK // NH                    # 16 head dim
CONV_CO = CO - DV               # 128
N_CORES = 8
BL = B // N_CORES               # 2 images per core
HP = H + 2                      # 34 padded
PADHW = HP * HP                 # 1156
SCALE = float(D) ** -0.5        # 0.25
EXA = 128.0 / 0.6931471805599453 * SCALE
EXB = 16249.5
import os as _os_
DVE_KTS = set(int(v) for v in _os_.environ.get(
    "AACONV_DVE_KTS", "0,2,4,5,7").split(",") if v != "")
LG_BUFS = int(_os_.environ.get("AACONV_LG_BUFS", "3"))
MS_BUFS = int(_os_.environ.get("AACONV_MS_BUFS", "1"))
AV_BUFS = int(_os_.environ.get("AACONV_AV_BUFS", "1"))
F32R = mybir.dt.float32r
RECIP_EPS = 1e-6


def build_nc():
    nc = bacc.Bacc("TRN2", target_bir_lowering=False, debug=False,
                   num_devices=N_CORES)

    x_ext = nc.declare_dram_parameter("x", [BL, C, HW], F32, isOutput=False)
    convw_ext = nc.declare_dram_parameter("conv_w", [9, C, CONV_CO], F32, isOutput=False)
    convb_ext = nc.declare_dram_parameter("conv_b", [1, CONV_CO], F32, isOutput=False)
    qkvw_ext = nc.declare_dram_parameter("qkv_w", [2 * DK + DV, C], F32, isOutput=False)
    qkvb_ext = nc.declare_dram_parameter("qkv_b", [1, 2 * DK + DV], F32, isOutput=False)
    projw_ext = nc.declare_dram_parameter("proj_w", [DV, DV], F32, isOutput=False)
    projb_ext = nc.declare_dram_parameter("proj_b", [1, DV], F32, isOutput=False)
    out_ext = nc.declare_dram_parameter("out", [BL, CO, HW], F32, isOutput=True)

    with tile.TileContext(nc) as tc:
        with (
            tc.tile_pool(name="const", bufs=1) as constp,
            tc.tile_pool(name="stage", bufs=1) as stagep,
            tc.tile_pool(name="img", bufs=3) as imgp,
            tc.tile_pool(name="st", bufs=6) as stp,
            tc.tile_pool(name="psum", bufs=1, space="PSUM") as psp,
        ):
            # ---------------- weights ----------------
            ident = constp.tile([128, 128], F32)
            make_identity(nc, ident[:])

            # qkv weights: DMA natural [chan, c], PE-transpose to [c, chan],
            # scatter into zero-padded layouts.
            qkvw_sb = stagep.tile([128, 3 * C], F32)  # blk b at cols b*256
            for blk in range(3):
                nc.sync.dma_start(
                    qkvw_sb[:, blk * C:(blk + 1) * C],
                    qkvw_ext[blk * 128:(blk + 1) * 128, :],
                )

            import os as _os
            _reps = int(_os.environ.get("AACONV_BENCH_REPS", "1"))

            # wkq_pad: [128 c, ct*512 + tgt*256 + hh*128 + hp*32 + d] bf16, zero pad
            wkq_pad = constp.tile([128, 2 * 512], BF16)
            nc.gpsimd.memset(wkq_pad[:], 0.0)
            wvT = constp.tile([128, 2 * 128], BF16)
            for ct in range(2):
                for blk in range(3):
                    tps = psp.tile([128, 128], F32, tag="ms", bufs=MS_BUFS, name=f"tps_{ct}_{blk}")
                    nc.tensor.transpose(
                        tps[:], qkvw_sb[:, blk * C + ct * 128: blk * C + (ct + 1) * 128],
                        ident[:])
                    if blk < 2:
                        dst = wkq_pad[:, ct * 512 + blk * 256:
                                      ct * 512 + (blk + 1) * 256].rearrange(
                            "p (h d) -> p h d", d=32)[:, :, 0:16]
                        src = tps[:].rearrange("p (h d) -> p h d", d=16)
                        nc.vector.tensor_copy(dst, src)
                    else:
                        nc.vector.tensor_copy(
                            wvT[:, ct * 128:(ct + 1) * 128], tps[:])

            projw_sb = stagep.tile([128, 128], F32)
            nc.scalar.dma_start(projw_sb[:], projw_ext[:])
            # padded projT: rows 32hp+16+d = proj_w^T row (4hh+hp)*16+d, rest 0
            # (matches the attn_pad layout where attn lives at rows 32hp+16..32).
            # Column-scatter proj_w in free space first, then PE-transpose.
            projw_pad = stagep.tile([128, 2 * 128], F32)
            nc.gpsimd.memset(projw_pad[:], 0.0)
            for hh in range(2):
                nc.vector.tensor_copy(
                    projw_pad[:, hh * 128:(hh + 1) * 128].rearrange(
                        "p (a b) -> p a b", b=32)[:, :, 16:32],
                    projw_sb[:, 64 * hh:64 * (hh + 1)].rearrange(
                        "p (a b) -> p a b", b=16))
            projT_pad = constp.tile([128, 2 * 128], BF16)
            for hh in range(2):
                tps2 = psp.tile([128, 128], F32, tag="ms", bufs=MS_BUFS, name=f"tps2_{hh}")
                nc.tensor.transpose(
                    tps2[:], projw_pad[:, hh * 128:(hh + 1) * 128], ident[:])
                nc.vector.tensor_copy(projT_pad[:, hh * 128:(hh + 1) * 128],
                                      tps2[:])

            # biases as per-partition COLUMNS (partition-scattered by DMA),
            # folded into the PSUM evacuations as tensor_scalar adds - no
            # bias matmuls on the PE.
            # bcol_kq[:, tgt*2+hh]: rows hp*32+d = qkv_b[tgt*128+(hh*4+hp)*16+d]

            # v-bias broadcast to 128 partitions via PE (ones ⊗ bv)
            bv_f32 = stagep.tile([1, DV], F32)
            nc.gpsimd.dma_start(bv_f32[:], qkvb_ext[:, 2 * DK:])
            bv_bf = stagep.tile([1, DV], BF16)
            nc.vector.tensor_copy(bv_bf[:], bv_f32[:])
            ones_row = constp.tile([1, 128], BF16)
            nc.gpsimd.memset(ones_row[:], 1.0)
            pre_ps = psp.tile([128, 512], F32, tag="ms", bufs=MS_BUFS, name="pre_ps")
            nc.tensor.matmul(pre_ps[:, 384:512], ones_row[:], bv_bf[:],
                             start=True, stop=True, skip_group_check=True)
            bv_bc = constp.tile([128, 128], F32)
            nc.vector.tensor_copy(bv_bc[:], pre_ps[:, 384:512])

            # conv weights: natural [c, o] per tap, bf16. cols (ct*9+t)*128+o
            # One big DMA per ct half (src AP transposes t<->c), one big
            # bf16 downcast; issued on the ACT queue to keep SP free.
            wconv_f32 = stagep.tile([128, 2 * 9 * CONV_CO], F32)
            wconv = constp.tile([128, 2 * 9 * CONV_CO], BF16)
            for ct in range(2):
                blk = slice(ct * 9 * CONV_CO, (ct + 1) * 9 * CONV_CO)
                nc.scalar.dma_start(
                    wconv_f32[:, blk].rearrange("p (t o) -> p t o", t=9),
                    convw_ext[:, ct * 128:(ct + 1) * 128, :].rearrange(
                        "t c o -> c t o"))
                nc.vector.tensor_copy(wconv[:, blk], wconv_f32[:, blk])

            # --- bias columns + quadmask, built via PE (no SBUF-writing DMAs)
            # bias ROWS first (free-dim scatters, DVE-legal), then a K=1
            # matmul against ones[1,1] turns each row into a column.
            qkvb_sb = stagep.tile([1, 2 * DK + DV], F32)
            nc.gpsimd.dma_start(qkvb_sb[:], qkvb_ext[:])
            brow_pad = constp.tile([1, 512], BF16)
            nc.gpsimd.memset(brow_pad[:], 0.0)
            for tgt in range(2):
                for hh in range(2):
                    nc.vector.tensor_copy(
                        brow_pad[0:1, (tgt * 2 + hh) * 128:
                                 (tgt * 2 + hh + 1) * 128].rearrange(
                            "p (a b) -> p a b", b=32)[:, :, 0:16],
                        qkvb_sb[0:1, tgt * DK + 64 * hh: tgt * DK + 64 * (hh + 1)
                                ].rearrange("p (a b) -> p a b", b=16))
            convb_f32 = stagep.tile([1, CONV_CO], F32)
            nc.scalar.dma_start(convb_f32[:], convb_ext[:])
            convb_row = constp.tile([1, CONV_CO], BF16)
            nc.vector.tensor_copy(convb_row[:], convb_f32[:])
            projb_f32 = stagep.tile([1, DV], F32)
            nc.gpsimd.dma_start(projb_f32[:], projb_ext[:])
            projb_row = constp.tile([1, DV], BF16)
            nc.vector.tensor_copy(projb_row[:], projb_f32[:])
            ones11 = constp.tile([1, 1], BF16)
            nc.gpsimd.memset(ones11[:], 1.0)
            for blk in range(4):
                nc.tensor.matmul(pre_ps[:, blk:blk + 1],
                                 brow_pad[0:1, blk * 128:(blk + 1) * 128],
                                 ones11[0:1, :], start=True, stop=True,
                                 skip_group_check=True)
            nc.tensor.matmul(pre_ps[:, 4:5], convb_row[0:1, :], ones11[0:1, :],
                             start=True, stop=True, skip_group_check=True)
            nc.tensor.matmul(pre_ps[:, 5:6], projb_row[0:1, :], ones11[0:1, :],
                             start=True, stop=True, skip_group_check=True)
            bias_cols = constp.tile([128, 6], F32)
            nc.vector.tensor_copy(bias_cols[:], pre_ps[:, 0:6])
            # selmask [128,128] f32 (used as f32r): row 32r has ones on cols
            # 32r..32r+32 -> matmul(selmask, rrec) broadcasts quadrant-row
            # 32r of rrec to all 32 output rows of quadrant r. Junk rows of
            # rrec hit zero weights (rrec kept finite by a max-clamp).
            selmask = constp.tile([128, 128], BF16)
            nc.vector.memset(selmask[:], 0.0)
            for r in range(4):
                nc.vector.memset(selmask[32 * r:32 * r + 1, 32 * r:32 * (r + 1)],
                                 1.0)

            # ---------------- per image, software-pipelined ----------------
            # Stages are emitted in an interleaved order so the PE always has
            # conv / next-image kq work queued while ACT+DVE chew on the
            # current round's exp backlog:
            #   pre(0), [round r; conv chunk r]*, in(i+2), round 3, pre(i+1),
            #   proj(i), ...
            PADW = PADHW + 36   # room for the last conv chunk's shifted reads
            CHUNKS = ((0, 15), (15, 15), (30, 2))
            n_imgs = BL * _reps
            imgs = {}
            pend = [None]    # deferred normalize tail (global across images)

            def normalize(av_sb, avc, rrec, rrec_bf, attn_pad, sl, slh, tag):
                # av rows per quadrant hp: 32hp = den, +1..16 = pad,
                # +16..32 = unnormalized attn. Clamp away from zero so
                # rrec is finite on junk rows, then broadcast quadrant
                # den-row reciprocals via a single selmask matmul (bf16).
                nc.vector.tensor_scalar(avc[:, sl], av_sb[:], RECIP_EPS,
                                        None, ALU.max)
                nc.vector.reciprocal_approx_fast(rrec[:, sl], avc[:, sl])
                nc.vector.tensor_copy(rrec_bf[:, sl], rrec[:, sl])
                rdps = psp.tile([128, 512], F32, tag="ms", bufs=MS_BUFS,
                                name=f"rdps_{tag}")
                nc.tensor.matmul(rdps[:], selmask[:], rrec_bf[:, sl],
                                 start=True, stop=True)
                nc.vector.tensor_mul(attn_pad[:, slh], av_sb[:], rdps[:])

            def stage_in(img):
                xin = imgp.tile([128, 2 * HW], F32, tag="xin", name=f"xin_{img}")
                for ct in range(2):
                    nc.sync.dma_start(
                        xin[:, ct * HW:(ct + 1) * HW],
                        x_ext[img % BL, ct * 128:(ct + 1) * 128, :])
                x_bf = imgp.tile([128, 2 * HW], BF16, tag="xbf", name=f"xbf_{img}")
                # image 0's compaction is on the critical path: use DVE
                # (fast); later images convert on the otherwise-idle Pool.
                eng = nc.vector if img == 0 else nc.gpsimd
                eng.tensor_copy(x_bf[:], xin[:])
                imgs[img] = {"x_bf": x_bf}

            def stage_pre(img):
                s = imgs[img]
                x_bf = s["x_bf"]
                # zero-padded 34x34 layout for the conv, filled via DVE
                xpad = imgp.tile([128, 2 * PADW], BF16, tag="xpad",
                                 name=f"xpad_{img}")
                nc.gpsimd.memset(xpad[:], 0.0)
                for ct in range(2):
                    nc.vector.tensor_copy(
                        xpad[:, ct * PADW: ct * PADW + PADHW].rearrange(
                            "p (h w) -> p h w", h=HP)[:, 1:33, 1:33],
                        x_bf[:, ct * HW:(ct + 1) * HW].rearrange(
                            "p (h w) -> p h w", h=H))

                # ---- K_pad / Q_pad ----
                k_pad = imgp.tile([128, 2 * HW], BF16, tag="kpad", name=f"kpad_{img}")
                q_pad = imgp.tile([128, 2 * HW], BF16, tag="qpad", name=f"qpad_{img}")
                for hh in range(2):
                    for tgt, dst in ((0, k_pad), (1, q_pad)):
                        for qn in range(2):
                            kqps = psp.tile([128, 512], F32, tag="ms", bufs=MS_BUFS,
                                            name=f"kqps_{img}_{tgt}_{hh}_{qn}")
                            for ct in range(2):
                                nc.tensor.matmul(
                                    kqps[:],
                                    wkq_pad[:, ct * 512 + tgt * 256 + hh * 128:
                                            ct * 512 + tgt * 256 + (hh + 1) * 128],
                                    x_bf[:, ct * HW + qn * 512:
                                         ct * HW + (qn + 1) * 512],
                                    start=(ct == 0), stop=(ct == 1))
                            nc.vector.tensor_scalar(
                                dst[:, hh * HW + qn * 512:
                                    hh * HW + (qn + 1) * 512], kqps[:],
                                bias_cols[:, tgt * 2 + hh: tgt * 2 + hh + 1],
                                None, ALU.add)

                # ---- V^T with ones column, 32-stride padded blocks ----
                # vt_aug block (hh,kt) at cols (hh*8+kt)*128 + hp*32 +
                #   [0 = ones, 1:16 = zeros, 16:32 = V_h]  (M=32 AV matmuls
                #   write full PSUM quadrants; denominator lands on quadrant
                #   rows 32hp, attn on rows 32hp+16..32)
                vt_aug = imgp.tile([128, 2 * 8 * 128], BF16, tag="vtaug",
                                   name=f"vtaug_{img}")
                # pad value 1e-4 (not 0) keeps the reciprocal of pad rows
                # finite; proj weights for pad rows are exactly 0 so the
                # values never reach the output
                nc.gpsimd.memset(vt_aug[:], 1e-4)
                nc.gpsimd.memset(
                    vt_aug[:].rearrange("p (g d) -> p g d", d=32)[:, :, 0:1], 1.0)
                for kt in range(8):
                    vtps = psp.tile([128, 128], F32, tag="ms", bufs=MS_BUFS,
                                    name=f"vtps_{img}_{kt}")
                    for ct in range(2):
                        nc.tensor.matmul(
                            vtps[:],
                            x_bf[:, ct * HW + kt * 128: ct * HW + (kt + 1) * 128],
                            wvT[:, ct * 128:(ct + 1) * 128],
                            start=(ct == 0), stop=(ct == 1))
                    for hh in range(2):
                        base = (hh * 8 + kt) * 128
                        dst = vt_aug[:, base: base + 128].rearrange(
                            "p (h d) -> p h d", d=32)[:, :, 16:32]
                        src = vtps[:, hh * 64:(hh + 1) * 64].rearrange(
                            "p (h d) -> p h d", d=16)
                        bvb = bv_bc[:, hh * 64:(hh + 1) * 64].rearrange(
                            "p (h d) -> p h d", d=16)
                        nc.vector.tensor_add(dst, src, bvb)

                s["xpad"] = xpad
                s["k_pad"], s["q_pad"], s["vt_aug"] = k_pad, q_pad, vt_aug
                s["out_conv"] = imgp.tile([128, HW], F32, tag="oconv",
                                          name=f"oconv_{img}")
                s["attn_pad"] = imgp.tile([128, 2 * HW], BF16, tag="attnp",
                                          name=f"attnp_{img}")
                s["avc"] = imgp.tile([128, HW], F32, tag="avc", name=f"avc_{img}")
                s["rrec"] = imgp.tile([128, HW], F32, tag="rrec", name=f"rrec_{img}")
                s["rrec_bf"] = imgp.tile([128, HW], BF16, tag="rrecbf",
                                         name=f"rrecbf_{img}")

            def stage_conv(img, ci):
                s = imgs[img]
                r0, nr = CHUNKS[ci]
                n = (nr - 1) * HP + W          # chunk free size (<=512)
                cs = (r0 + 1) * HP + 1         # pad-flat offset of (r0, 0)
                cvps = psp.tile([128, 512], F32, tag="ms", bufs=MS_BUFS,
                                name=f"cvps_{img}_{r0}")
                for t in range(9):
                    dy, dx = t // 3, t % 3
                    sh = (dy - 1) * HP + (dx - 1)
                    for ct in range(2):
                        nc.tensor.matmul(
                            cvps[:, 0:n],
                            wconv[:, (ct * 9 + t) * 128:(ct * 9 + t + 1) * 128],
                            s["xpad"][:, ct * PADW + cs + sh:
                                      ct * PADW + cs + sh + n],
                            start=((t, ct) == (0, 0)), stop=((t, ct) == (8, 1)))
                nc.vector.tensor_scalar(
                    s["out_conv"][:, r0 * W:(r0 + nr) * W].rearrange(
                        "p (h w) -> p h w", h=nr),
                    cvps[:, 0:nr * HP].rearrange(
                        "p (h w) -> p h w", w=HP)[:, :, 0:W],
                    bias_cols[:, 4:5], None, ALU.add)
                if ci == 2:
                    nc.sync.dma_start(out_ext[img % BL, 0:CONV_CO, :],
                                      s["out_conv"][:])

            def stage_round(img, ridx):
                s = imgs[img]
                hh, qh = divmod(ridx, 2)
                k_pad, q_pad, vt_aug = s["k_pad"], s["q_pad"], s["vt_aug"]
                sl = slice(qh * 512, (qh + 1) * 512)
                slh = slice(hh * HW + qh * 512, hh * HW + (qh + 1) * 512)
                av = psp.tile([128, 512], F32, tag="av", bufs=AV_BUFS,
                              name=f"av_{img}_{hh}_{qh}")
                for kt in range(8):
                    lgs = []
                    for hg in range(2):
                        lg = psp.tile([128, 1024], F32, tag="lg", bufs=LG_BUFS,
                                      name=f"lg_{img}_{hh}_{qh}_{kt}_{hg}")
                        lgs.append(lg)
                        for j in range(2):
                            hp = 2 * hg + j
                            nc.tensor.matmul(
                                lg[:, j * 512:(j + 1) * 512],
                                k_pad[32 * hp:32 * hp + 16,
                                      hh * HW + kt * 128: hh * HW + (kt + 1) * 128],
                                q_pad[32 * hp:32 * hp + 16,
                                      hh * HW + qh * 512: hh * HW + (qh + 1) * 512],
                                start=True, stop=True,
                                tile_position=(32 * hp, 0))
                    sts = []
                    for hg in range(2):
                        st = stp.tile([128, 1024], BF16, tag="st",
                                      name=f"st_{img}_{hh}_{qh}_{kt}_{hg}")
                        sts.append(st)
                        if hg == 0 or kt not in DVE_KTS:
                            nc.scalar.activation(st[:], lgs[hg][:], AF.Exp,
                                                 scale=SCALE)
                        else:
                            nc.vector.tensor_scalar(
                                st[:].bitcast(I16), lgs[hg][:],
                                EXA, EXB, ALU.mult, ALU.add)
                    for hg in range(2):
                        for j in range(2):
                            hp = 2 * hg + j
                            base = (hh * 8 + kt) * 128 + 32 * hp
                            nc.tensor.matmul(
                                av[32 * hp:32 * hp + 32, :],
                                vt_aug[:, base: base + 32],
                                sts[hg][:, j * 512:(j + 1) * 512],
                                start=(kt == 0), stop=(kt == 7),
                                skip_group_check=True,
                                tile_position=(0, 32 * hp))
                    if kt == 0 and pend[0] is not None:
                        normalize(*pend[0])
                        pend[0] = None
                # evacuate av to SBUF right away so the PSUM bank
                # frees; defer the rest of the normalize.
                av_sb = imgp.tile([128, 512], F32, tag="avsb",
                                  name=f"avsb_{img}_{hh}_{qh}")
                nc.vector.tensor_copy(av_sb[:], av[:])
                pend[0] = (av_sb, s["avc"], s["rrec"], s["rrec_bf"],
                           s["attn_pad"], sl, slh, f"{img}_{hh}_{qh}")

            def stage_proj(img):
                s = imgs[img]
                if pend[0] is not None:
                    normalize(*pend[0])
                    pend[0] = None
                out_proj = imgp.tile([128, HW], F32, tag="oproj",
                                     name=f"oproj_{img}")
                for qn in range(2):
                    projps = psp.tile([128, 512], F32, tag="ms", bufs=MS_BUFS,
                                      name=f"projps_{img}_{qn}")
                    for hh in range(2):
                        nc.tensor.matmul(
                            projps[:],
                            projT_pad[:, hh * 128:(hh + 1) * 128],
                            s["attn_pad"][:, hh * HW + qn * 512:
                                          hh * HW + (qn + 1) * 512],
                            start=(hh == 0), stop=(hh == 1))
                    nc.vector.tensor_scalar(
                        out_proj[:, qn * 512:(qn + 1) * 512], projps[:],
                        bias_cols[:, 5:6], None, ALU.add)
                nc.sync.dma_start(out_ext[img % BL, CONV_CO:, :], out_proj[:])

            for i in range(min(2, n_imgs)):
                stage_in(i)
            stage_pre(0)
            for img in range(n_imgs):
                for r in range(3):
                    stage_round(img, r)
                    stage_conv(img, r)
                if img + 2 < n_imgs:
                    stage_in(img + 2)
                stage_round(img, 3)
                if img + 1 < n_imgs:
                    stage_pre(img + 1)
                stage_proj(img)
                del imgs[img]

    return nc


_NC = None


def _get_nc():
    global _NC
    if _NC is None:
        _NC = build_nc()
        _NC.compile()
    return _NC


def kernel(**inputs):
    from concourse.bass_utils import run_bass_kernel_spmd

    nc = _get_nc()
    x = np.asarray(inputs["x"], np.float32).reshape(B, C, HW)
    conv_w = np.ascontiguousarray(np.asarray(inputs["conv_w"], np.float32).reshape(9, C, CONV_CO))
    conv_b = np.ascontiguousarray(np.asarray(inputs["conv_b"], np.float32).reshape(1, CONV_CO))
    qkv_w = np.ascontiguousarray(np.asarray(inputs["qkv_w"], np.float32))
    qkv_b = np.ascontiguousarray(np.asarray(inputs["qkv_b"], np.float32).reshape(1, 2 * DK + DV))
    proj_w = np.ascontiguousarray(np.asarray(inputs["proj_w"], np.float32))
    proj_b = np.ascontiguousarray(np.asarray(inputs["proj_b"], np.float32).reshape(1, DV))

    in_maps = []
    for i in range(N_CORES):
        in_maps.append({
            "x": np.ascontiguousarray(x[i * BL:(i + 1) * BL]),
            "conv_w": conv_w, "conv_b": conv_b,
            "qkv_w": qkv_w, "qkv_b": qkv_b,
            "proj_w": proj_w, "proj_b": proj_b,
        })
    res = run_bass_kernel_spmd(nc, in_maps, core_ids=list(range(N_CORES)))
    outs = [np.asarray(res.results[i]["out"]).reshape(BL, CO, H, W)
            for i in range(N_CORES)]
    return np.concatenate(outs, axis=0).astype(np.float32)


if __name__ == "__main__":
    nc = build_nc()
    nc.compile()
    print("built ok; instructions:", len(nc.inst_map))

